# revision 86
# baseline (speedup 1.0000x reference)
"""Multi-head attention with 2D RoPE on 8 Trainium2 NeuronCores.

Problem (hardcoded): B=8, L=1024, EMB=768, 12 heads x 64 dim, 2D RoPE
(x/y tables of length 32, base 100), softmax attention, output projection.

Sharding: data-parallel over batch — one batch element per core, no
collectives.

Per-core kernel structure (v3 — PE-roofline oriented):

    qT/kT = (W/pair)^T @ embT in fp32r, rope via elementwise cos/ssh plus a
        16-lane swap as a PE matmul (128x128 permutation, in-place PSUM)
    per (pair, q-half 512, key-tile j):
        sT = kT^T @ qT        [128 keys, 2 heads x 512 q], tile_position row
                              packing, fp32r, ap 512 (output-bandwidth floor)
        expT = exp(sT) bf16   (ACT, no max-subtraction: |scores| <~ 6)
        AV FLIPPED: stationary = expT [128k x 128q] slices (ldweights is
            free), moving = v tile bf16 [128 x 64] -> av2[q, d] in PSUM.
            49.9k PE cycles vs 98.3k for the moving-expT orientation.
        sums = expT^T @ ones  (ap-1 matmuls into a shared [128, 96] bank)
        All sub-bank accumulation regions share one PSUM zero-region:
        banks are DVE-memset to 0 and every accumulate uses start=False +
        skip_group_check (a regular matmul's start=True zeroes its whole
        2KB zero-region on HW and would wipe sibling regions).
    normalize: r = 1/sums per (q, head) on DVE; DVE scale-copies
        av2 -> avsb bf16 (per-partition scalar = per-query, native)
    transpose avsb [q, chan] -> avT [chan, q] via PE transpose matmuls
        (bf16 identity; XBAR DMA transpose gives corrupt data), DVE drains
        PSUM->SBUF. Transposes are deferred into the next half's j-loop so
        the in-order PE queue never waits on the DVE normalize chain.
    out = attnout @ Wp(bf16) + bp, K=128 accumulation over pairs; the last
        half runs in two 256-query groups so the final projection overlaps
        the attention epilogue.

Engine budget (TimelineSim model): PE ~305k cycles @2.4GHz ~= 127us is the
bottleneck; ACT exp ~102us; DVE (rope/normalize) ~99us. Next-pair
projections interleave into the j-loops (matmul and rope parts emitted
separately), AV emission lags scores by 2 j-tiles, and startup loads embT
in column halves so the first projections start DMA-paced.
"""

import numpy as np

import concourse.bass as bass
import concourse.mybir as mybir
import concourse.tile as tile
from concourse import bacc
from concourse.bass import ts
from concourse.bass_utils import run_bass_kernel_spmd

F32 = mybir.dt.float32
F32R = mybir.dt.float32r
BF16 = mybir.dt.bfloat16
AF = mybir.ActivationFunctionType

HEAD_NUM = 12
EMB = 768
HEAD = 64
L = 1024
B = 8
X_SIZE = 32
Y_SIZE = 32
BASE = 100.0
N_CORES = 8

KT = EMB // 128   # 6 contraction tiles over channels
NJ = L // 128     # 8 key tiles
NPAIR = HEAD_NUM // 2  # 6 head pairs


def _round_f32r(x):
    """Round fp32 to FP22 (e8m13, drop 10 mantissa LSBs, RNE)."""
    v = np.ascontiguousarray(x, dtype=np.float32).view(np.uint32).copy()
    v = v + (np.uint32(0x1FF) + ((v >> np.uint32(10)) & np.uint32(1)))
    v &= np.uint32(0xFFFFFC00)
    return v.view(np.float32)


def _tables_np(pos_len, d, base=BASE):
    inv_freq = 1.0 / base ** (np.arange(0, d, 2, dtype=np.float32) / d)
    freqs = np.outer(np.arange(pos_len, dtype=np.float32), inv_freq)
    freqs = np.concatenate([freqs, freqs], axis=-1)
    return np.sin(freqs).astype(np.float32), np.cos(freqs).astype(np.float32)


def _rope_coeffs(pos):
    """cos128/ssh128: [128, L] elementwise RoPE coefficients, 2 heads deep.

    Row layout per 64-row head block: rows 0:32 x-part, rows 32:64 y-part.
    ssh is the sin table pre-shifted/negated so that
        rope(q) = q * cos128 + R128 @ (q * ssh128)
    where R128 swaps 16-row halves within each 32-row block.
    """
    sx, cx = _tables_np(X_SIZE, HEAD // 2)
    sy, cy = _tables_np(Y_SIZE, HEAD // 2)
    px, py = pos[:, 0], pos[:, 1]
    cosxT = cx[px].T  # [32, L]
    cosyT = cy[py].T
    sinxT = sx[px].T
    sinyT = sy[py].T

    def shift(s):
        out = np.empty_like(s)
        out[0:16] = s[16:32]
        out[16:32] = -s[0:16]
        return out

    cos64 = np.concatenate([cosxT, cosyT], axis=0)          # [64, L]
    ssh64 = np.concatenate([shift(sinxT), shift(sinyT)], axis=0)
    cos128 = np.concatenate([cos64, cos64], axis=0).astype(np.float32)
    ssh128 = np.concatenate([ssh64, ssh64], axis=0).astype(np.float32)
    return np.ascontiguousarray(cos128), np.ascontiguousarray(ssh128)


def _r128():
    r32 = np.zeros((32, 32), dtype=np.float32)
    for d in range(16):
        r32[d, d + 16] = 1.0
        r32[d + 16, d] = 1.0
    return np.kron(np.eye(4, dtype=np.float32), r32)


def build_nc(debug=False):
    nc = bacc.Bacc()
    embT = nc.declare_dram_parameter("embT", [EMB, L], F32R, isOutput=False)
    wqs = nc.declare_dram_parameter("wqs", [NPAIR, 128, EMB], F32R,
                                    isOutput=False)
    wks = nc.declare_dram_parameter("wks", [NPAIR, 128, EMB], F32R,
                                    isOutput=False)
    wv = nc.declare_dram_parameter("wv", [EMB, EMB], F32R, isOutput=False)
    wp = nc.declare_dram_parameter("wp", [EMB, EMB], BF16, isOutput=False)
    bp = nc.declare_dram_parameter("bp", [1, EMB], F32, isOutput=False)
    cos = nc.declare_dram_parameter("cos", [128, L], F32, isOutput=False)
    ssh = nc.declare_dram_parameter("ssh", [128, L], F32, isOutput=False)
    r128 = nc.declare_dram_parameter("r128", [128, 128], F32R, isOutput=False)
    ident = nc.declare_dram_parameter("ident", [128, 128], BF16,
                                      isOutput=False)
    out = nc.declare_dram_parameter("out", [L, EMB], F32, isOutput=True)
    if debug:
        d_avT = nc.declare_dram_parameter("d_avT", [NPAIR, 128, NJ, 128],
                                          BF16, isOutput=True)

    with tile.TileContext(nc) as tc:
        with (
            tc.tile_pool(name="const", bufs=1) as p_const,
            tc.tile_pool(name="vaug", bufs=1) as p_vaug,
            tc.tile_pool(name="persist", bufs=1) as p_per,
            tc.tile_pool(name="wsl", bufs=2) as p_wsl,
            tc.tile_pool(name="qk", bufs=2) as p_qk,
            tc.tile_pool(name="tmp", bufs=2) as p_tmp,
            tc.tile_pool(name="exp", bufs=8) as p_exp,
            tc.tile_pool(name="avsb", bufs=3) as p_avsb,
            tc.tile_pool(name="rsb", bufs=3) as p_rsb,
            tc.tile_pool(name="outp", bufs=6) as p_out,
            tc.tile_pool(name="opart", bufs=4) as p_opart,
            tc.tile_pool(name="big", bufs=2, space="PSUM") as ps_big,
            tc.tile_pool(name="qp", bufs=2, space="PSUM") as ps_qp,
            tc.tile_pool(name="av", bufs=1, space="PSUM") as ps_av,
            tc.tile_pool(name="sum", bufs=1, space="PSUM") as ps_sum,
        ):
            avT_t = [p_per.tile([128, NJ, 128], BF16, tag=f"avT{p}",
                                name=f"avT{p}") for p in range(NPAIR)]

            # weight slices are pre-swizzled on the host to the exact
            # [128, KT*128] SBUF layout, so each load is one contiguous DMA
            def load_wslice(w_dram, pair, wtag):
                wsl = p_wsl.tile([128, EMB], F32R, tag=wtag,
                                 name=f"wsl{wtag}{pair}")
                nc.sync.dma_start(wsl[:], w_dram[pair])
                return wsl

            # ---- startup loads. embT goes in column halves so the first
            # projection chunks start as soon as their half arrives.
            PRE_Q0 = load_wslice(wqs, 0, "q")

            # PE warmup during the DMA head keeps the HAM clock-gate warm
            wu = p_const.tile([128, 512], F32, tag="warm")
            nc.gpsimd.memset(wu[:], 0.0)
            wup = ps_qp.tile([128, 512], F32, tag="qp", name="warmps")
            for _ in range(2):
                nc.tensor.matmul(wup[0:64, :], wu[:, 0:64], wu[:],
                                 start=True, stop=True)

            embT_t = [p_per.tile([128, L], F32R, tag=f"embT{k}",
                                 name=f"embTt{k}") for k in range(KT)]
            for k in range(KT):
                eng = nc.sync if k % 2 == 0 else nc.scalar
                eng.dma_start(embT_t[k][:, 0:512], embT[ts(k, 128), 0:512])
            cos_t = p_const.tile([128, L], F32, tag="cos")
            ssh_t = p_const.tile([128, L], F32, tag="ssh")
            r_t = p_const.tile([128, 128], F32R, tag="r128")
            nc.sync.dma_start(cos_t[:], cos[:])
            nc.scalar.dma_start(ssh_t[:], ssh[:])
            nc.sync.dma_start(r_t[:], r128[:])
            PRE_K0 = load_wslice(wks, 0, "k")
            for k in range(KT):
                eng = nc.sync if k % 2 == 0 else nc.scalar
                eng.dma_start(embT_t[k][:, 512:L], embT[ts(k, 128), 512:L])
            PRE_Q1 = load_wslice(wqs, 1, "q")
            PRE_K1 = load_wslice(wks, 1, "k")
            wv_t = [p_per.tile([128, EMB], F32R, tag=f"wvp{k}",
                               name=f"wvt{k}") for k in range(KT)]
            for k in range(KT):
                nc.sync.dma_start(wv_t[k][:], wv[ts(k, 128), :])
            # bf16 identity for PE transposes
            id_t = p_const.tile([128, 128], BF16, tag="ident")
            nc.sync.dma_start(id_t[:], ident[:])
            wp_t = [p_per.tile([128, EMB], BF16, tag=f"wpp{k}",
                               name=f"wpt{k}") for k in range(KT)]
            for k in range(KT):
                nc.sync.dma_start(wp_t[k][:], wp[ts(k, 128), :])
            bpb_t = p_const.tile([128, EMB], F32, tag="bpb")
            nc.sync.dma_start(bpb_t[:], bp[:].to_broadcast((128, EMB)))

            # ones moving-vector for the ap-1 softmax-denominator matmuls
            ones_mv = p_const.tile([128, 1], BF16, tag="ones")
            nc.gpsimd.memset(ones_mv[:], 1.0)
            # dummy activation during the DMA head pulls the 1.3us exp
            # table load off the first real exp's critical path
            edum = p_const.tile([128, 32], BF16, tag="edum")
            nc.scalar.activation(edum[:], wu[:, 0:32], AF.Exp)

            # one shared PSUM bank of [128, 1] denominator accumulators,
            # region col = pair*16 + half*8 + qs*2 + head; zeroed once
            sums_ps = ps_sum.tile([128, 96], F32, tag="sums")
            nc.vector.memset(sums_ps[:], 0.0)

            vaug_t = [p_vaug.tile([128, EMB], BF16, tag=f"vaug{j}",
                                  name=f"vaug{j}")
                      for j in range(NJ)]

            def rope_into(dst, src, c0):
                """dst[:, c0:c0+512] = rope(src chunk); the rotation matmul
                overwrites the PSUM chunk in place."""
                t_s = p_tmp.tile([128, 512], F32R, tag="ts")
                t_c = p_tmp.tile([128, 512], F32, tag="tc")
                nc.vector.tensor_mul(t_s[:], src, ssh_t[:, c0:c0 + 512])
                nc.vector.tensor_mul(t_c[:], src, cos_t[:, c0:c0 + 512])
                nc.tensor.matmul(src, r_t[:], t_s[:], start=True, stop=True)
                nc.vector.tensor_add(dst[:, c0:c0 + 512], t_c[:], src)

            def proj_mm(psum, wsl, c0):
                for k in range(KT):
                    nc.tensor.matmul(
                        psum,
                        wsl[:, ts(k, 128)],
                        embT_t[k][:, c0:c0 + 512],
                        start=(k == 0), stop=(k == KT - 1),
                    )

            # rope'd projection, emission split into 3-matmul parts and a
            # rope part so the PE work spreads evenly over the j-loop
            def make_proj(pair, wtag, wsl):
                dst = p_qk.tile([128, L], F32R, tag=wtag,
                                name=f"{wtag}T{pair}")
                qps = {}

                def mm(ci, part):
                    if part == 0:
                        qps[ci] = ps_qp.tile([128, 512], F32, tag="qp",
                                             name=f"qp{wtag}{pair}{ci}")
                    qp = qps[ci]
                    c0 = ci * 512
                    for k in (0, 1, 2) if part == 0 else (3, 4, 5):
                        nc.tensor.matmul(
                            qp[:],
                            wsl[:, ts(k, 128)],
                            embT_t[k][:, c0:c0 + 512],
                            start=(k == 0), stop=(k == KT - 1),
                        )

                def rope(ci):
                    rope_into(dst, qps[ci][:], ci * 512)
                return dst, mm, rope

            # v projection for one key tile -> vaug[j] (bf16); channel
            # layout already matches the flipped-AV moving operand
            def project_v(j):
                vp = ps_big.tile([128, L], F32, tag="big", name=f"vp{j}")
                for c0, c1 in ((0, 512), (512, 768)):
                    for k in range(KT):
                        nc.tensor.matmul(
                            vp[:, c0:c1],
                            embT_t[k][:, ts(j, 128)],
                            wv_t[k][:, c0:c1],
                            start=(k == 0), stop=(k == KT - 1),
                        )
                nc.vector.tensor_copy(vaug_t[j][:], vp[:, 0:EMB])

            # final projection for one 128-query tile (+bias, store)
            def fp_qtile(qt):
                fp = ps_big.tile([128, L], F32, tag="big", name=f"fp{qt}")
                for c0, c1 in ((0, 512), (512, 768)):
                    for pp in range(NPAIR):
                        nc.tensor.matmul(
                            fp[:, c0:c1],
                            avT_t[pp][:, qt],
                            wp_t[pp][:, c0:c1],
                            start=(pp == 0), stop=(pp == NPAIR - 1),
                        )
                o_sb = p_out.tile([128, EMB], F32, tag="osb",
                                  name=f"osb{qt}")
                nc.vector.tensor_add(o_sb[:], fp[:, 0:EMB], bpb_t[:])
                oeng = nc.sync if qt % 2 == 0 else nc.scalar
                oeng.dma_start(out[ts(qt, 128), :], o_sb[:])

            # split final projection: pairs 0-3 (+bias) pre-accumulated
            # early into SBUF, pair 4 added in place, pair 5 at the tail
            oparts = {}

            def fp_partial(qt):
                fp = ps_big.tile([128, L], F32, tag="big", name=f"fpp{qt}")
                for c0, c1 in ((0, 512), (512, 768)):
                    for pp in range(3):
                        nc.tensor.matmul(
                            fp[:, c0:c1],
                            avT_t[pp][:, qt],
                            wp_t[pp][:, c0:c1],
                            start=(pp == 0), stop=(pp == 2),
                        )
                o_part = p_opart.tile([128, EMB], F32, tag="opart",
                                      name=f"opart{qt}")
                nc.vector.tensor_add(o_part[:], fp[:, 0:EMB], bpb_t[:])
                oparts[qt] = o_part

            def fp_update(qt, plo, phi):
                fp = ps_big.tile([128, L], F32, tag="big",
                                 name=f"fpu{qt}_{plo}")
                for c0, c1 in ((0, 512), (512, 768)):
                    for pp in range(plo, phi + 1):
                        nc.tensor.matmul(
                            fp[:, c0:c1],
                            avT_t[pp][:, qt],
                            wp_t[pp][:, c0:c1],
                            start=(pp == plo), stop=(pp == phi),
                        )
                nc.vector.tensor_add(oparts[qt][:], oparts[qt][:],
                                     fp[:, 0:EMB])

            def fp_finish(qt):
                fp = ps_big.tile([128, L], F32, tag="big", name=f"fpf{qt}")
                for c0, c1 in ((0, 512), (512, 768)):
                    nc.tensor.matmul(
                        fp[:, c0:c1],
                        avT_t[NPAIR - 1][:, qt],
                        wp_t[NPAIR - 1][:, c0:c1],
                        start=True, stop=True,
                    )
                o_sb = p_out.tile([128, EMB], F32, tag="osb",
                                  name=f"osbf{qt}")
                nc.vector.tensor_add(o_sb[:], fp[:, 0:EMB], oparts[qt][:])
                oeng = nc.sync if qt % 2 == 0 else nc.scalar
                oeng.dma_start(out[ts(qt, 128), :], o_sb[:])

            # ---- attention building blocks -------------------------------

            def emit_scores_exp(pair, half, j, qT, kT, qlo, qw):
                """scores + exp for key tile j over queries [qlo, qlo+qw)
                per head (qw <= 512). Head h0 lands in sAB bank 0, h1 in
                bank 1 (a start=True matmul zeroes its whole 2KB region, so
                the two heads must not share a bank)."""
                sAB = ps_big.tile([128, L], F32, tag="big",
                                  name=f"s{pair}_{half}_{qlo}_{j}")
                for hh in range(2):
                    p0 = 64 * hh
                    nc.tensor.matmul(
                        sAB[:, 512 * hh:512 * hh + qw],
                        kT[p0:p0 + 64, ts(j, 128)],
                        qT[p0:p0 + 64, qlo:qlo + qw],
                        start=True, stop=True,
                        tile_position=(p0, 0),
                    )
                expt = p_exp.tile([128, L], BF16, tag="expt",
                                  name=f"e{pair}_{half}_{qlo}_{j}")
                if qw == 512:
                    nc.scalar.activation(expt[:], sAB[:], AF.Exp)
                else:
                    nc.scalar.activation(
                        expt[:, 0:2 * qw],
                        sAB[:].rearrange("p (h c) -> p h c", h=2)[:, :, 0:qw],
                        AF.Exp)
                return expt

            def emit_av(pair, half, av2, j, expt, qsbase, nqs):
                """flipped AV + denominator matmuls; expt holds 2 heads x
                nqs*128 queries packed [h0 | h1]."""
                for qs in range(nqs):
                    for hh in range(2):
                        e_sl = expt[:, (qsbase + hh * nqs + qs) * 0 +
                                    nqs * 128 * hh + 128 * qs:
                                    nqs * 128 * hh + 128 * qs + 128]
                        g = qsbase + qs
                        nc.tensor.matmul(
                            av2[:, g * 128 + hh * 64:g * 128 + hh * 64 + 64],
                            e_sl,
                            vaug_t[j][:, (2 * pair + hh) * 64:
                                      (2 * pair + hh) * 64 + 64],
                            start=False, stop=(j == NJ - 1),
                            skip_group_check=True,
                        )
                        sc = pair * 16 + half * 8 + g * 2 + hh
                        nc.tensor.matmul(
                            sums_ps[:, sc:sc + 1],
                            e_sl,
                            ones_mv[:],
                            start=False, stop=(j == NJ - 1),
                            skip_group_check=True,
                        )

            def emit_norm(pair, half, av2, qsbase, nqs, on_act):
                """reciprocal + scale-copy av2 -> avsb bf16 for qtiles
                [qsbase, qsbase+nqs) of this half."""
                soff = pair * 16 + half * 8 + qsbase * 2
                r_sb = p_rsb.tile([128, 8], F32, tag="rsb",
                                  name=f"r{pair}_{half}_{qsbase}")
                nc.vector.reciprocal_approx_fast(
                    r_sb[:, 0:2 * nqs], sums_ps[:, soff:soff + 2 * nqs])
                avsb = p_avsb.tile([128, 4, 128], BF16, tag="avsb",
                                   name=f"avsb{pair}_{half}_{qsbase}")
                if not on_act:
                    # one broadcast multiply: r repeats over each 64-wide
                    # head block (free-dim stride-0 read)
                    rb = r_sb[:, 0:2 * nqs][:, :, None].to_broadcast(
                        (128, 2 * nqs, 64))
                    nc.vector.tensor_mul(
                        avsb[:].rearrange("p a b -> p (a b)")
                        [:, 0:nqs * 128].rearrange("p (a b) -> p a b", b=64),
                        av2[:, qsbase * 128:qsbase * 128 + nqs * 128]
                        .rearrange("p (a b) -> p a b", b=64),
                        rb)
                else:
                    for qs in range(nqs):
                        g = qsbase + qs
                        for hh in range(2):
                            nc.scalar.mul(
                                avsb[:, qs, hh * 64:hh * 64 + 64],
                                av2[:, g * 128 + hh * 64:
                                    g * 128 + hh * 64 + 64],
                                r_sb[:, qs * 2 + hh:qs * 2 + hh + 1])
                return avsb

            def make_transposes(pair, half, avsb, qsbase, nqs, on_act):
                def run():
                    tp = ps_big.tile([128, L], F32, tag="big",
                                     name=f"tp{pair}_{half}_{qsbase}")
                    for qs in range(nqs):
                        sub = tp[:, 128 * qs:128 * qs + 64].bitcast(BF16)
                        nc.tensor.transpose(sub, avsb[:, qs, :], id_t[:])
                        dst = avT_t[pair][:, 4 * half + qsbase + qs, :]
                        if on_act:
                            nc.scalar.copy(dst, sub)
                        else:
                            nc.vector.tensor_copy(dst, sub)
                return run

            # ---- pair 0 startup: chunk-ordered projections + special
            # half-0 (scores first, v-projections once wv arrives) --------
            qT = p_qk.tile([128, L], F32R, tag="q", name="qT0")
            kT = p_qk.tile([128, L], F32R, tag="k", name="kT0")
            qp_q0 = ps_qp.tile([128, 512], F32, tag="qp", name="qp_q0c0")
            proj_mm(qp_q0[:], PRE_Q0, 0)
            kp0 = ps_qp.tile([128, 512], F32, tag="qp", name="kp0")
            proj_mm(kp0[:], PRE_K0, 0)
            rope_into(qT, qp_q0[:], 0)
            rope_into(kT, kp0[:], 0)

            av2 = ps_av.tile([128, 512], F32, tag="av2", name="av2_0_0")
            nc.vector.memset(av2[:], 0.0)

            # half-0 scores for key tiles 0..3 (kT chunk 0 only)
            p0h0_exps = []
            for j in range(4):
                p0h0_exps.append(emit_scores_exp(0, 0, j, qT, kT, 0, 512))
            # chunk-1 projections (paced by the embT second-half DMAs)
            qp_q1 = ps_qp.tile([128, 512], F32, tag="qp", name="qp_q0c1")
            proj_mm(qp_q1[:], PRE_Q0, 512)
            kp1 = ps_qp.tile([128, 512], F32, tag="qp", name="kp1")
            proj_mm(kp1[:], PRE_K0, 512)
            rope_into(qT, qp_q1[:], 512)
            rope_into(kT, kp1[:], 512)
            for j in range(4, NJ):
                p0h0_exps.append(emit_scores_exp(0, 0, j, qT, kT, 0, 512))
            # pair-1 q projection fills the gap until wv arrives
            qT_n, qn_mm, qn_rope = make_proj(1, "q", PRE_Q1)
            kT_n, kn_mm, kn_rope = make_proj(1, "k", PRE_K1)
            qn_mm(0, 0)
            qn_mm(0, 1)
            qn_rope(0)
            qn_mm(1, 0)
            qn_mm(1, 1)
            qn_rope(1)
            # v projections + deferred AV (lag 1 behind the v matmuls so
            # the DVE bf16 copy has drained)
            for j in range(NJ):
                project_v(j)
                if j >= 1:
                    emit_av(0, 0, av2, j - 1, p0h0_exps[j - 1], 0, 4)
            emit_av(0, 0, av2, NJ - 1, p0h0_exps[NJ - 1], 0, 4)
            p0h0_exps = None

            av2_next = ps_av.tile([128, 512], F32, tag="av2", name="av2_0_1")
            nc.vector.memset(av2_next[:], 0.0)
            avsb = emit_norm(0, 0, av2, 0, 4, False)
            pending_norm = make_transposes(0, 0, avsb, 0, 4, False)
            av2 = av2_next

            # The last two AV emissions and the normalize of each half are
            # carried into the NEXT half's j-loop, so the PE queue never
            # stalls on the final exps of a half (each stall would also
            # cost a ~3us p-state re-ramp).
            carry = []       # (pair, half, av2, j, expt) tuples
            norm_ctx = None  # (pair, half, av2) awaiting normalize

            halves = [(p, h) for p in range(NPAIR) for h in (0, 1)][1:]
            for pair, half in halves:
                last = (pair == NPAIR - 1 and half == 1)
                if half == 0 and pair + 1 < NPAIR:
                    # pair+1 weight slices + projection closures; the q
                    # chunks are emitted in this half's j-loop, the k
                    # chunks in the next one
                    wsl_q = load_wslice(wqs, pair + 1, "q")
                    wsl_k = load_wslice(wks, pair + 1, "k")
                    qT_n, qn_mm, qn_rope = make_proj(pair + 1, "q", wsl_q)
                    kT_n, kn_mm, kn_rope = make_proj(pair + 1, "k", wsl_k)

                pend = []
                for j in range(NJ):
                    if j == 0 and carry:
                        emit_av(*carry.pop(0), 0, 4)
                    if j == 1:
                        while carry:
                            emit_av(*carry.pop(0), 0, 4)
                        if norm_ctx is not None:
                            p_, h_, a_ = norm_ctx
                            avsb = emit_norm(p_, h_, a_, 0, 4, False)
                            pending_norm = make_transposes(p_, h_, avsb,
                                                           0, 4, False)
                            norm_ctx = None
                    if len(pend) >= 2:
                        emit_av(pair, half, av2, *pend.pop(0), 0, 4)
                    expt = emit_scores_exp(pair, half, j, qT, kT,
                                           512 * half, 512)
                    pend.append((j, expt))
                    if j == 3 and pending_norm is not None:
                        pending_norm()
                        pending_norm = None
                    # next-pair projection chunks (q during half 0,
                    # k during half 1), spread across the loop
                    if not last and pair + 1 < NPAIR:
                        mmf, ropef = ((qn_mm, qn_rope) if half == 0
                                      else (kn_mm, kn_rope))
                        if j == 1:
                            mmf(0, 0)
                        elif j == 2:
                            mmf(0, 1)
                        elif j == 3:
                            ropef(0)
                        elif j == 4:
                            mmf(1, 0)
                        elif j == 5:
                            mmf(1, 1)
                        elif j == 6:
                            ropef(1)
                    # the projection-free final pair pre-accumulates the
                    # half-1 output projections: pairs 0-3 during j0-3
                    # (avT[0..3] complete), pair 4 once its transposes
                    # flushed at j==3
                    if pair == NPAIR - 1 and half == 0:
                        if j < 4:
                            fp_partial(4 + j)
                        else:
                            fp_update(j, 3, 4)
                if not last:
                    carry = [(pair, half, av2, jj, ee) for jj, ee in pend]
                    norm_ctx = (pair, half, av2)
                    av2_next = ps_av.tile([128, 512], F32, tag="av2",
                                          name=f"av2n_{pair}_{half}")
                    nc.vector.memset(av2_next[:], 0.0)
                    av2 = av2_next
                    if half == 1 and pair + 1 < NPAIR:
                        qT, kT = qT_n, kT_n
                else:
                    # full projections for qt0-3 run after the exp stream
                    # is fully emitted, so the deferred AVs never wait on
                    # ACT; qt4-7 need only the pair-5 finish matmuls
                    for qt in range(4):
                        fp_qtile(qt)
                    for item in pend:
                        emit_av(pair, half, av2, *item, 0, 4)
                    avsb = emit_norm(pair, half, av2, 0, 4, False)
                    make_transposes(pair, half, avsb, 0, 4, True)()
                    for qt in range(4, NJ):
                        fp_finish(qt)

            if debug:
                for p in range(NPAIR):
                    nc.sync.dma_start(d_avT[p], avT_t[p][:])

    nc.finalize()
    return nc


_NC_CACHE = {}


def _get_nc(variant=None):
    if variant not in _NC_CACHE:
        _NC_CACHE[variant] = build_nc(debug=(variant == "debug"))
    return _NC_CACHE[variant]


def kernel(emb, pos, Wq, Wk, Wv, Wp, bp, _trace=False, _cores=N_CORES,
           _debug=False):
    import ml_dtypes

    emb = np.asarray(emb, dtype=np.float32)
    pos = np.asarray(pos)
    Wq_s = _round_f32r(np.asarray(Wq, dtype=np.float32) * (HEAD ** -0.5))
    Wk_r = _round_f32r(np.asarray(Wk, dtype=np.float32))
    Wv_r = _round_f32r(np.asarray(Wv, dtype=np.float32))
    Wp_b = np.asarray(Wp, dtype=np.float32).astype(ml_dtypes.bfloat16)
    bp2 = np.asarray(bp, dtype=np.float32).reshape(1, EMB)

    cos128, ssh128 = _rope_coeffs(np.asarray(pos))
    r128 = _r128()
    ident = np.eye(128, dtype=np.float32).astype(ml_dtypes.bfloat16)

    def swizzle(w):
        # [EMB, EMB] -> [NPAIR, 128, KT*128]: slice pair columns, gather
        # row t*128+p to partition p, k-tile-major free layout
        return np.ascontiguousarray(
            w.reshape(KT, 128, NPAIR, 128).transpose(2, 1, 0, 3)
            .reshape(NPAIR, 128, EMB))

    nc = _get_nc("debug" if _debug else None)
    wqs = swizzle(Wq_s)
    wks = swizzle(Wk_r)
    in_maps = []
    for b in range(_cores):
        in_maps.append({
            "embT": _round_f32r(emb[b].T),
            "wqs": wqs, "wks": wks, "wv": Wv_r, "wp": Wp_b, "bp": bp2,
            "cos": cos128, "ssh": ssh128, "r128": r128, "ident": ident,
        })
    res = run_bass_kernel_spmd(nc, in_maps, list(range(_cores)), trace=_trace)
    out = np.stack([res.results[b]["out"] for b in range(_cores)], axis=0)
    if _debug:
        return out, res.results
    if _trace:
        return out, res
    return out


# revision 87
# speedup vs baseline: 1.0007x; 1.0007x over previous
"""Multi-head attention with 2D RoPE on 8 Trainium2 NeuronCores.

Problem (hardcoded): B=8, L=1024, EMB=768, 12 heads x 64 dim, 2D RoPE
(x/y tables of length 32, base 100), softmax attention, output projection.

Sharding: data-parallel over batch — one batch element per core, no
collectives.

Per-core kernel structure (v3 — PE-roofline oriented):

    qT/kT = (W/pair)^T @ embT in fp32r, rope via elementwise cos/ssh plus a
        16-lane swap as a PE matmul (128x128 permutation, in-place PSUM)
    per (pair, q-half 512, key-tile j):
        sT = kT^T @ qT        [128 keys, 2 heads x 512 q], tile_position row
                              packing, fp32r, ap 512 (output-bandwidth floor)
        expT = exp(sT) bf16   (ACT, no max-subtraction: |scores| <~ 6)
        AV FLIPPED: stationary = expT [128k x 128q] slices (ldweights is
            free), moving = v tile bf16 [128 x 64] -> av2[q, d] in PSUM.
            49.9k PE cycles vs 98.3k for the moving-expT orientation.
        sums = expT^T @ ones  (ap-1 matmuls into a shared [128, 96] bank)
        All sub-bank accumulation regions share one PSUM zero-region:
        banks are DVE-memset to 0 and every accumulate uses start=False +
        skip_group_check (a regular matmul's start=True zeroes its whole
        2KB zero-region on HW and would wipe sibling regions).
    normalize: r = 1/sums per (q, head) on DVE; DVE scale-copies
        av2 -> avsb bf16 (per-partition scalar = per-query, native)
    transpose avsb [q, chan] -> avT [chan, q] via PE transpose matmuls
        (bf16 identity; XBAR DMA transpose gives corrupt data), DVE drains
        PSUM->SBUF. Transposes are deferred into the next half's j-loop so
        the in-order PE queue never waits on the DVE normalize chain.
    out = attnout @ Wp(bf16) + bp, K=128 accumulation over pairs; the last
        half runs in two 256-query groups so the final projection overlaps
        the attention epilogue.

Engine budget (TimelineSim model): PE ~305k cycles @2.4GHz ~= 127us is the
bottleneck; ACT exp ~102us; DVE (rope/normalize) ~99us. Next-pair
projections interleave into the j-loops (matmul and rope parts emitted
separately), AV emission lags scores by 2 j-tiles, and startup loads embT
in column halves so the first projections start DMA-paced.
"""

import numpy as np

import concourse.bass as bass
import concourse.mybir as mybir
import concourse.tile as tile
from concourse import bacc
from concourse.bass import ts
from concourse.bass_utils import run_bass_kernel_spmd

F32 = mybir.dt.float32
F32R = mybir.dt.float32r
BF16 = mybir.dt.bfloat16
AF = mybir.ActivationFunctionType

HEAD_NUM = 12
EMB = 768
HEAD = 64
L = 1024
B = 8
X_SIZE = 32
Y_SIZE = 32
BASE = 100.0
N_CORES = 8

KT = EMB // 128   # 6 contraction tiles over channels
NJ = L // 128     # 8 key tiles
NPAIR = HEAD_NUM // 2  # 6 head pairs


def _round_f32r(x):
    """Round fp32 to FP22 (e8m13, drop 10 mantissa LSBs, RNE)."""
    v = np.ascontiguousarray(x, dtype=np.float32).view(np.uint32).copy()
    v = v + (np.uint32(0x1FF) + ((v >> np.uint32(10)) & np.uint32(1)))
    v &= np.uint32(0xFFFFFC00)
    return v.view(np.float32)


def _tables_np(pos_len, d, base=BASE):
    inv_freq = 1.0 / base ** (np.arange(0, d, 2, dtype=np.float32) / d)
    freqs = np.outer(np.arange(pos_len, dtype=np.float32), inv_freq)
    freqs = np.concatenate([freqs, freqs], axis=-1)
    return np.sin(freqs).astype(np.float32), np.cos(freqs).astype(np.float32)


def _rope_coeffs(pos):
    """cos128/ssh128: [128, L] elementwise RoPE coefficients, 2 heads deep.

    Row layout per 64-row head block: rows 0:32 x-part, rows 32:64 y-part.
    ssh is the sin table pre-shifted/negated so that
        rope(q) = q * cos128 + R128 @ (q * ssh128)
    where R128 swaps 16-row halves within each 32-row block.
    """
    sx, cx = _tables_np(X_SIZE, HEAD // 2)
    sy, cy = _tables_np(Y_SIZE, HEAD // 2)
    px, py = pos[:, 0], pos[:, 1]
    cosxT = cx[px].T  # [32, L]
    cosyT = cy[py].T
    sinxT = sx[px].T
    sinyT = sy[py].T

    def shift(s):
        out = np.empty_like(s)
        out[0:16] = s[16:32]
        out[16:32] = -s[0:16]
        return out

    cos64 = np.concatenate([cosxT, cosyT], axis=0)          # [64, L]
    ssh64 = np.concatenate([shift(sinxT), shift(sinyT)], axis=0)
    cos128 = np.concatenate([cos64, cos64], axis=0).astype(np.float32)
    ssh128 = np.concatenate([ssh64, ssh64], axis=0).astype(np.float32)
    return np.ascontiguousarray(cos128), np.ascontiguousarray(ssh128)


def _r128():
    r32 = np.zeros((32, 32), dtype=np.float32)
    for d in range(16):
        r32[d, d + 16] = 1.0
        r32[d + 16, d] = 1.0
    return np.kron(np.eye(4, dtype=np.float32), r32)


def build_nc(debug=False):
    nc = bacc.Bacc()
    embT = nc.declare_dram_parameter("embT", [EMB, L], F32R, isOutput=False)
    wqs = nc.declare_dram_parameter("wqs", [NPAIR, 128, EMB], F32R,
                                    isOutput=False)
    wks = nc.declare_dram_parameter("wks", [NPAIR, 128, EMB], F32R,
                                    isOutput=False)
    wv = nc.declare_dram_parameter("wv", [EMB, EMB], F32R, isOutput=False)
    wp = nc.declare_dram_parameter("wp", [EMB, EMB], BF16, isOutput=False)
    bp = nc.declare_dram_parameter("bp", [1, EMB], F32, isOutput=False)
    cos = nc.declare_dram_parameter("cos", [128, L], F32, isOutput=False)
    ssh = nc.declare_dram_parameter("ssh", [128, L], F32, isOutput=False)
    r128 = nc.declare_dram_parameter("r128", [128, 128], F32R, isOutput=False)
    ident = nc.declare_dram_parameter("ident", [128, 128], BF16,
                                      isOutput=False)
    out = nc.declare_dram_parameter("out", [L, EMB], F32, isOutput=True)
    if debug:
        d_avT = nc.declare_dram_parameter("d_avT", [NPAIR, 128, NJ, 128],
                                          BF16, isOutput=True)

    with tile.TileContext(nc) as tc:
        with (
            tc.tile_pool(name="const", bufs=1) as p_const,
            tc.tile_pool(name="vaug", bufs=1) as p_vaug,
            tc.tile_pool(name="persist", bufs=1) as p_per,
            tc.tile_pool(name="wsl", bufs=2) as p_wsl,
            tc.tile_pool(name="qk", bufs=2) as p_qk,
            tc.tile_pool(name="tmp", bufs=2) as p_tmp,
            tc.tile_pool(name="exp", bufs=8) as p_exp,
            tc.tile_pool(name="avsb", bufs=3) as p_avsb,
            tc.tile_pool(name="rsb", bufs=3) as p_rsb,
            tc.tile_pool(name="outp", bufs=6) as p_out,
            tc.tile_pool(name="opart", bufs=4) as p_opart,
            tc.tile_pool(name="big", bufs=2, space="PSUM") as ps_big,
            tc.tile_pool(name="qp", bufs=2, space="PSUM") as ps_qp,
            tc.tile_pool(name="av", bufs=1, space="PSUM") as ps_av,
            tc.tile_pool(name="sum", bufs=1, space="PSUM") as ps_sum,
        ):
            avT_t = [p_per.tile([128, NJ, 128], BF16, tag=f"avT{p}",
                                name=f"avT{p}") for p in range(NPAIR)]

            # weight slices are pre-swizzled on the host to the exact
            # [128, KT*128] SBUF layout, so each load is one contiguous DMA
            def load_wslice(w_dram, pair, wtag):
                wsl = p_wsl.tile([128, EMB], F32R, tag=wtag,
                                 name=f"wsl{wtag}{pair}")
                nc.sync.dma_start(wsl[:], w_dram[pair])
                return wsl

            # ---- startup loads. embT goes in column halves so the first
            # projection chunks start as soon as their half arrives.
            PRE_Q0 = load_wslice(wqs, 0, "q")

            # PE warmup during the DMA head keeps the HAM clock-gate warm
            wu = p_const.tile([128, 512], F32, tag="warm")
            nc.gpsimd.memset(wu[:], 0.0)
            wup = ps_qp.tile([128, 512], F32, tag="qp", name="warmps")
            for _ in range(2):
                nc.tensor.matmul(wup[0:64, :], wu[:, 0:64], wu[:],
                                 start=True, stop=True)

            embT_t = [p_per.tile([128, L], F32R, tag=f"embT{k}",
                                 name=f"embTt{k}") for k in range(KT)]
            for k in range(KT):
                eng = nc.sync if k % 2 == 0 else nc.scalar
                eng.dma_start(embT_t[k][:, 0:512], embT[ts(k, 128), 0:512])
            cos_t = p_const.tile([128, L], F32, tag="cos")
            ssh_t = p_const.tile([128, L], F32, tag="ssh")
            r_t = p_const.tile([128, 128], F32R, tag="r128")
            nc.sync.dma_start(cos_t[:], cos[:])
            nc.scalar.dma_start(ssh_t[:], ssh[:])
            nc.sync.dma_start(r_t[:], r128[:])
            PRE_K0 = load_wslice(wks, 0, "k")
            for k in range(KT):
                eng = nc.sync if k % 2 == 0 else nc.scalar
                eng.dma_start(embT_t[k][:, 512:L], embT[ts(k, 128), 512:L])
            PRE_Q1 = load_wslice(wqs, 1, "q")
            PRE_K1 = load_wslice(wks, 1, "k")
            wv_t = [p_per.tile([128, EMB], F32R, tag=f"wvp{k}",
                               name=f"wvt{k}") for k in range(KT)]
            for k in range(KT):
                nc.sync.dma_start(wv_t[k][:], wv[ts(k, 128), :])
            # bf16 identity for PE transposes
            id_t = p_const.tile([128, 128], BF16, tag="ident")
            nc.sync.dma_start(id_t[:], ident[:])
            wp_t = [p_per.tile([128, EMB], BF16, tag=f"wpp{k}",
                               name=f"wpt{k}") for k in range(KT)]
            for k in range(KT):
                nc.sync.dma_start(wp_t[k][:], wp[ts(k, 128), :])
            bpb_t = p_const.tile([128, EMB], F32, tag="bpb")
            nc.sync.dma_start(bpb_t[:], bp[:].to_broadcast((128, EMB)))

            # ones moving-vector for the ap-1 softmax-denominator matmuls
            ones_mv = p_const.tile([128, 1], BF16, tag="ones")
            nc.gpsimd.memset(ones_mv[:], 1.0)

            # one shared PSUM bank of [128, 1] denominator accumulators,
            # region col = pair*16 + half*8 + qs*2 + head; zeroed once
            sums_ps = ps_sum.tile([128, 96], F32, tag="sums")
            nc.vector.memset(sums_ps[:], 0.0)

            vaug_t = [p_vaug.tile([128, EMB], BF16, tag=f"vaug{j}",
                                  name=f"vaug{j}")
                      for j in range(NJ)]

            def rope_into(dst, src, c0):
                """dst[:, c0:c0+512] = rope(src chunk); the rotation matmul
                overwrites the PSUM chunk in place."""
                t_s = p_tmp.tile([128, 512], F32R, tag="ts")
                t_c = p_tmp.tile([128, 512], F32, tag="tc")
                nc.vector.tensor_mul(t_s[:], src, ssh_t[:, c0:c0 + 512])
                nc.vector.tensor_mul(t_c[:], src, cos_t[:, c0:c0 + 512])
                nc.tensor.matmul(src, r_t[:], t_s[:], start=True, stop=True)
                nc.vector.tensor_add(dst[:, c0:c0 + 512], t_c[:], src)

            def proj_mm(psum, wsl, c0):
                for k in range(KT):
                    nc.tensor.matmul(
                        psum,
                        wsl[:, ts(k, 128)],
                        embT_t[k][:, c0:c0 + 512],
                        start=(k == 0), stop=(k == KT - 1),
                    )

            # rope'd projection, emission split into 3-matmul parts and a
            # rope part so the PE work spreads evenly over the j-loop
            def make_proj(pair, wtag, wsl):
                dst = p_qk.tile([128, L], F32R, tag=wtag,
                                name=f"{wtag}T{pair}")
                qps = {}

                def mm(ci, part):
                    if part == 0:
                        qps[ci] = ps_qp.tile([128, 512], F32, tag="qp",
                                             name=f"qp{wtag}{pair}{ci}")
                    qp = qps[ci]
                    c0 = ci * 512
                    for k in (0, 1, 2) if part == 0 else (3, 4, 5):
                        nc.tensor.matmul(
                            qp[:],
                            wsl[:, ts(k, 128)],
                            embT_t[k][:, c0:c0 + 512],
                            start=(k == 0), stop=(k == KT - 1),
                        )

                def rope(ci):
                    rope_into(dst, qps[ci][:], ci * 512)
                return dst, mm, rope

            # v projection for one key tile -> vaug[j] (bf16); channel
            # layout already matches the flipped-AV moving operand
            def project_v(j):
                vp = ps_big.tile([128, L], F32, tag="big", name=f"vp{j}")
                for c0, c1 in ((0, 512), (512, 768)):
                    for k in range(KT):
                        nc.tensor.matmul(
                            vp[:, c0:c1],
                            embT_t[k][:, ts(j, 128)],
                            wv_t[k][:, c0:c1],
                            start=(k == 0), stop=(k == KT - 1),
                        )
                nc.vector.tensor_copy(vaug_t[j][:], vp[:, 0:EMB])

            # final projection for one 128-query tile (+bias, store)
            def fp_qtile(qt):
                fp = ps_big.tile([128, L], F32, tag="big", name=f"fp{qt}")
                for c0, c1 in ((0, 512), (512, 768)):
                    for pp in range(NPAIR):
                        nc.tensor.matmul(
                            fp[:, c0:c1],
                            avT_t[pp][:, qt],
                            wp_t[pp][:, c0:c1],
                            start=(pp == 0), stop=(pp == NPAIR - 1),
                        )
                o_sb = p_out.tile([128, EMB], F32, tag="osb",
                                  name=f"osb{qt}")
                nc.vector.tensor_add(o_sb[:], fp[:, 0:EMB], bpb_t[:])
                oeng = nc.sync if qt % 2 == 0 else nc.scalar
                oeng.dma_start(out[ts(qt, 128), :], o_sb[:])

            # split final projection: pairs 0-3 (+bias) pre-accumulated
            # early into SBUF, pair 4 added in place, pair 5 at the tail
            oparts = {}

            def fp_partial(qt):
                fp = ps_big.tile([128, L], F32, tag="big", name=f"fpp{qt}")
                for c0, c1 in ((0, 512), (512, 768)):
                    for pp in range(3):
                        nc.tensor.matmul(
                            fp[:, c0:c1],
                            avT_t[pp][:, qt],
                            wp_t[pp][:, c0:c1],
                            start=(pp == 0), stop=(pp == 2),
                        )
                o_part = p_opart.tile([128, EMB], F32, tag="opart",
                                      name=f"opart{qt}")
                nc.vector.tensor_add(o_part[:], fp[:, 0:EMB], bpb_t[:])
                oparts[qt] = o_part

            def fp_update(qt, plo, phi):
                fp = ps_big.tile([128, L], F32, tag="big",
                                 name=f"fpu{qt}_{plo}")
                for c0, c1 in ((0, 512), (512, 768)):
                    for pp in range(plo, phi + 1):
                        nc.tensor.matmul(
                            fp[:, c0:c1],
                            avT_t[pp][:, qt],
                            wp_t[pp][:, c0:c1],
                            start=(pp == plo), stop=(pp == phi),
                        )
                nc.vector.tensor_add(oparts[qt][:], oparts[qt][:],
                                     fp[:, 0:EMB])

            def fp_finish(qt):
                fp = ps_big.tile([128, L], F32, tag="big", name=f"fpf{qt}")
                for c0, c1 in ((0, 512), (512, 768)):
                    nc.tensor.matmul(
                        fp[:, c0:c1],
                        avT_t[NPAIR - 1][:, qt],
                        wp_t[NPAIR - 1][:, c0:c1],
                        start=True, stop=True,
                    )
                o_sb = p_out.tile([128, EMB], F32, tag="osb",
                                  name=f"osbf{qt}")
                nc.vector.tensor_add(o_sb[:], fp[:, 0:EMB], oparts[qt][:])
                oeng = nc.sync if qt % 2 == 0 else nc.scalar
                oeng.dma_start(out[ts(qt, 128), :], o_sb[:])

            # ---- attention building blocks -------------------------------

            def emit_scores_exp(pair, half, j, qT, kT, qlo, qw):
                """scores + exp for key tile j over queries [qlo, qlo+qw)
                per head (qw <= 512). Head h0 lands in sAB bank 0, h1 in
                bank 1 (a start=True matmul zeroes its whole 2KB region, so
                the two heads must not share a bank)."""
                sAB = ps_big.tile([128, L], F32, tag="big",
                                  name=f"s{pair}_{half}_{qlo}_{j}")
                for hh in range(2):
                    p0 = 64 * hh
                    nc.tensor.matmul(
                        sAB[:, 512 * hh:512 * hh + qw],
                        kT[p0:p0 + 64, ts(j, 128)],
                        qT[p0:p0 + 64, qlo:qlo + qw],
                        start=True, stop=True,
                        tile_position=(p0, 0),
                    )
                expt = p_exp.tile([128, L], BF16, tag="expt",
                                  name=f"e{pair}_{half}_{qlo}_{j}")
                if qw == 512:
                    nc.scalar.activation(expt[:], sAB[:], AF.Exp)
                else:
                    nc.scalar.activation(
                        expt[:, 0:2 * qw],
                        sAB[:].rearrange("p (h c) -> p h c", h=2)[:, :, 0:qw],
                        AF.Exp)
                return expt

            def emit_av(pair, half, av2, j, expt, qsbase, nqs):
                """flipped AV + denominator matmuls; expt holds 2 heads x
                nqs*128 queries packed [h0 | h1]."""
                for qs in range(nqs):
                    for hh in range(2):
                        e_sl = expt[:, (qsbase + hh * nqs + qs) * 0 +
                                    nqs * 128 * hh + 128 * qs:
                                    nqs * 128 * hh + 128 * qs + 128]
                        g = qsbase + qs
                        nc.tensor.matmul(
                            av2[:, g * 128 + hh * 64:g * 128 + hh * 64 + 64],
                            e_sl,
                            vaug_t[j][:, (2 * pair + hh) * 64:
                                      (2 * pair + hh) * 64 + 64],
                            start=False, stop=(j == NJ - 1),
                            skip_group_check=True,
                        )
                        sc = pair * 16 + half * 8 + g * 2 + hh
                        nc.tensor.matmul(
                            sums_ps[:, sc:sc + 1],
                            e_sl,
                            ones_mv[:],
                            start=False, stop=(j == NJ - 1),
                            skip_group_check=True,
                        )

            def emit_norm(pair, half, av2, qsbase, nqs, on_act):
                """reciprocal + scale-copy av2 -> avsb bf16 for qtiles
                [qsbase, qsbase+nqs) of this half."""
                soff = pair * 16 + half * 8 + qsbase * 2
                r_sb = p_rsb.tile([128, 8], F32, tag="rsb",
                                  name=f"r{pair}_{half}_{qsbase}")
                nc.vector.reciprocal_approx_fast(
                    r_sb[:, 0:2 * nqs], sums_ps[:, soff:soff + 2 * nqs])
                avsb = p_avsb.tile([128, 4, 128], BF16, tag="avsb",
                                   name=f"avsb{pair}_{half}_{qsbase}")
                if not on_act:
                    # one broadcast multiply: r repeats over each 64-wide
                    # head block (free-dim stride-0 read)
                    rb = r_sb[:, 0:2 * nqs][:, :, None].to_broadcast(
                        (128, 2 * nqs, 64))
                    nc.vector.tensor_mul(
                        avsb[:].rearrange("p a b -> p (a b)")
                        [:, 0:nqs * 128].rearrange("p (a b) -> p a b", b=64),
                        av2[:, qsbase * 128:qsbase * 128 + nqs * 128]
                        .rearrange("p (a b) -> p a b", b=64),
                        rb)
                else:
                    for qs in range(nqs):
                        g = qsbase + qs
                        for hh in range(2):
                            nc.scalar.mul(
                                avsb[:, qs, hh * 64:hh * 64 + 64],
                                av2[:, g * 128 + hh * 64:
                                    g * 128 + hh * 64 + 64],
                                r_sb[:, qs * 2 + hh:qs * 2 + hh + 1])
                return avsb

            def make_transposes(pair, half, avsb, qsbase, nqs, on_act):
                def run():
                    tp = ps_big.tile([128, L], F32, tag="big",
                                     name=f"tp{pair}_{half}_{qsbase}")
                    for qs in range(nqs):
                        sub = tp[:, 128 * qs:128 * qs + 64].bitcast(BF16)
                        nc.tensor.transpose(sub, avsb[:, qs, :], id_t[:])
                        dst = avT_t[pair][:, 4 * half + qsbase + qs, :]
                        if on_act:
                            nc.scalar.copy(dst, sub)
                        else:
                            nc.vector.tensor_copy(dst, sub)
                return run

            # ---- pair 0 startup: chunk-ordered projections + special
            # half-0 (scores first, v-projections once wv arrives) --------
            qT = p_qk.tile([128, L], F32R, tag="q", name="qT0")
            kT = p_qk.tile([128, L], F32R, tag="k", name="kT0")
            qp_q0 = ps_qp.tile([128, 512], F32, tag="qp", name="qp_q0c0")
            proj_mm(qp_q0[:], PRE_Q0, 0)
            kp0 = ps_qp.tile([128, 512], F32, tag="qp", name="kp0")
            proj_mm(kp0[:], PRE_K0, 0)
            rope_into(qT, qp_q0[:], 0)
            rope_into(kT, kp0[:], 0)

            av2 = ps_av.tile([128, 512], F32, tag="av2", name="av2_0_0")
            nc.vector.memset(av2[:], 0.0)

            # half-0 scores for key tiles 0..3 (kT chunk 0 only)
            p0h0_exps = []
            for j in range(4):
                p0h0_exps.append(emit_scores_exp(0, 0, j, qT, kT, 0, 512))
            # chunk-1 projections (paced by the embT second-half DMAs)
            qp_q1 = ps_qp.tile([128, 512], F32, tag="qp", name="qp_q0c1")
            proj_mm(qp_q1[:], PRE_Q0, 512)
            kp1 = ps_qp.tile([128, 512], F32, tag="qp", name="kp1")
            proj_mm(kp1[:], PRE_K0, 512)
            rope_into(qT, qp_q1[:], 512)
            rope_into(kT, kp1[:], 512)
            for j in range(4, NJ):
                p0h0_exps.append(emit_scores_exp(0, 0, j, qT, kT, 0, 512))
            # pair-1 q projection fills the gap until wv arrives
            qT_n, qn_mm, qn_rope = make_proj(1, "q", PRE_Q1)
            kT_n, kn_mm, kn_rope = make_proj(1, "k", PRE_K1)
            qn_mm(0, 0)
            qn_mm(0, 1)
            qn_rope(0)
            qn_mm(1, 0)
            qn_mm(1, 1)
            qn_rope(1)
            # v projections + deferred AV (lag 1 behind the v matmuls so
            # the DVE bf16 copy has drained)
            for j in range(NJ):
                project_v(j)
                if j >= 1:
                    emit_av(0, 0, av2, j - 1, p0h0_exps[j - 1], 0, 4)
            emit_av(0, 0, av2, NJ - 1, p0h0_exps[NJ - 1], 0, 4)
            p0h0_exps = None

            av2_next = ps_av.tile([128, 512], F32, tag="av2", name="av2_0_1")
            nc.vector.memset(av2_next[:], 0.0)
            avsb = emit_norm(0, 0, av2, 0, 4, False)
            pending_norm = make_transposes(0, 0, avsb, 0, 4, False)
            av2 = av2_next

            # The last two AV emissions and the normalize of each half are
            # carried into the NEXT half's j-loop, so the PE queue never
            # stalls on the final exps of a half (each stall would also
            # cost a ~3us p-state re-ramp).
            carry = []       # (pair, half, av2, j, expt) tuples
            norm_ctx = None  # (pair, half, av2) awaiting normalize

            halves = [(p, h) for p in range(NPAIR) for h in (0, 1)][1:]
            for pair, half in halves:
                last = (pair == NPAIR - 1 and half == 1)
                if half == 0 and pair + 1 < NPAIR:
                    # pair+1 weight slices + projection closures; the q
                    # chunks are emitted in this half's j-loop, the k
                    # chunks in the next one
                    wsl_q = load_wslice(wqs, pair + 1, "q")
                    wsl_k = load_wslice(wks, pair + 1, "k")
                    qT_n, qn_mm, qn_rope = make_proj(pair + 1, "q", wsl_q)
                    kT_n, kn_mm, kn_rope = make_proj(pair + 1, "k", wsl_k)

                pend = []
                for j in range(NJ):
                    if j == 0 and carry:
                        emit_av(*carry.pop(0), 0, 4)
                    if j == 1:
                        while carry:
                            emit_av(*carry.pop(0), 0, 4)
                        if norm_ctx is not None:
                            p_, h_, a_ = norm_ctx
                            avsb = emit_norm(p_, h_, a_, 0, 4, False)
                            pending_norm = make_transposes(p_, h_, avsb,
                                                           0, 4, False)
                            norm_ctx = None
                    if len(pend) >= 2:
                        emit_av(pair, half, av2, *pend.pop(0), 0, 4)
                    expt = emit_scores_exp(pair, half, j, qT, kT,
                                           512 * half, 512)
                    pend.append((j, expt))
                    if j == 3 and pending_norm is not None:
                        pending_norm()
                        pending_norm = None
                    # next-pair projection chunks (q during half 0,
                    # k during half 1), spread across the loop
                    if not last and pair + 1 < NPAIR:
                        mmf, ropef = ((qn_mm, qn_rope) if half == 0
                                      else (kn_mm, kn_rope))
                        if j == 1:
                            mmf(0, 0)
                        elif j == 2:
                            mmf(0, 1)
                        elif j == 3:
                            ropef(0)
                        elif j == 4:
                            mmf(1, 0)
                        elif j == 5:
                            mmf(1, 1)
                        elif j == 6:
                            ropef(1)
                    # the projection-free final pair pre-accumulates the
                    # half-1 output projections: pairs 0-3 during j0-3
                    # (avT[0..3] complete), pair 4 once its transposes
                    # flushed at j==3
                    if pair == NPAIR - 1 and half == 0:
                        if j < 4:
                            fp_partial(4 + j)
                        else:
                            fp_update(j, 3, 4)
                if not last:
                    carry = [(pair, half, av2, jj, ee) for jj, ee in pend]
                    norm_ctx = (pair, half, av2)
                    av2_next = ps_av.tile([128, 512], F32, tag="av2",
                                          name=f"av2n_{pair}_{half}")
                    nc.vector.memset(av2_next[:], 0.0)
                    av2 = av2_next
                    if half == 1 and pair + 1 < NPAIR:
                        qT, kT = qT_n, kT_n
                else:
                    # full projections for qt0-3 run after the exp stream
                    # is fully emitted, so the deferred AVs never wait on
                    # ACT; qt4-7 need only the pair-5 finish matmuls
                    for qt in range(4):
                        fp_qtile(qt)
                    for item in pend:
                        emit_av(pair, half, av2, *item, 0, 4)
                    avsb = emit_norm(pair, half, av2, 0, 4, False)
                    make_transposes(pair, half, avsb, 0, 4, True)()
                    for qt in range(4, NJ):
                        fp_finish(qt)

            if debug:
                for p in range(NPAIR):
                    nc.sync.dma_start(d_avT[p], avT_t[p][:])

    nc.finalize()
    return nc


_NC_CACHE = {}


def _get_nc(variant=None):
    if variant not in _NC_CACHE:
        _NC_CACHE[variant] = build_nc(debug=(variant == "debug"))
    return _NC_CACHE[variant]


def kernel(emb, pos, Wq, Wk, Wv, Wp, bp, _trace=False, _cores=N_CORES,
           _debug=False):
    import ml_dtypes

    emb = np.asarray(emb, dtype=np.float32)
    pos = np.asarray(pos)
    Wq_s = _round_f32r(np.asarray(Wq, dtype=np.float32) * (HEAD ** -0.5))
    Wk_r = _round_f32r(np.asarray(Wk, dtype=np.float32))
    Wv_r = _round_f32r(np.asarray(Wv, dtype=np.float32))
    Wp_b = np.asarray(Wp, dtype=np.float32).astype(ml_dtypes.bfloat16)
    bp2 = np.asarray(bp, dtype=np.float32).reshape(1, EMB)

    cos128, ssh128 = _rope_coeffs(np.asarray(pos))
    r128 = _r128()
    ident = np.eye(128, dtype=np.float32).astype(ml_dtypes.bfloat16)

    def swizzle(w):
        # [EMB, EMB] -> [NPAIR, 128, KT*128]: slice pair columns, gather
        # row t*128+p to partition p, k-tile-major free layout
        return np.ascontiguousarray(
            w.reshape(KT, 128, NPAIR, 128).transpose(2, 1, 0, 3)
            .reshape(NPAIR, 128, EMB))

    nc = _get_nc("debug" if _debug else None)
    wqs = swizzle(Wq_s)
    wks = swizzle(Wk_r)
    in_maps = []
    for b in range(_cores):
        in_maps.append({
            "embT": _round_f32r(emb[b].T),
            "wqs": wqs, "wks": wks, "wv": Wv_r, "wp": Wp_b, "bp": bp2,
            "cos": cos128, "ssh": ssh128, "r128": r128, "ident": ident,
        })
    res = run_bass_kernel_spmd(nc, in_maps, list(range(_cores)), trace=_trace)
    out = np.stack([res.results[b]["out"] for b in range(_cores)], axis=0)
    if _debug:
        return out, res.results
    if _trace:
        return out, res
    return out
